# revision 18
# baseline (speedup 1.0000x reference)
"""Trainium2 Bass kernel for nn_FMG_6717328851807 (dense_transformer).

Reference computation (B=8, C=512, H=W=64, K=64, MEM=512, heads=8, d=64):
    q = Wq @ x            (1x1 conv)          -> [B,h,N,d], N = H*W = 4096
    k = Ft @ Wk.T, v = Ft @ Wv.T              -> [B,h,K,d]
    attn = softmax(q k^T / sqrt(d))           -> [B,h,N,K]
    out = attn @ v                            -> [B,h,N,d]
    y = x + Wp @ out + bp

Sharding: pure data-parallel over B - one batch element per NeuronCore,
no collectives. Within a core everything runs transposed (channels on
partitions, spatial N on the free dim) in NW=512-column chunks, and
every PE op is a full-width matmul:

    qT[C,N]    = Wq8.T @ x8            fp8e4m3 DoubleRow: 2 MMs of
                                       256-deep contraction per 128-row
                                       group (Wq pre-scaled by 64 on the
                                       host; the 1/64 folds into the
                                       softmax exp scale)
    kT, v      = bf16 setup matmuls, then repacked into BLOCK-DIAGONAL
                 [128,128] tiles per head pair so that
    scoresT    = kbd.T @ qT            one 128-wide MM per head pair
    expT       = exp(scoresT/(8*64))   ScalarE, bf16 out
    sums_bc    = blockones.T @ expT    one MM per pair computes the
                                       softmax denominator AND
                                       broadcasts it to all 64
                                       partitions of its head
                                       (blockones holds 1/64, so the
                                       reciprocal is pre-scaled for the
                                       fp8 out tile)
    rb         = recip_approx(sums_bc) DVE, fp32
    outT8      = po * rb               DVE multiply, written straight
                                       into the fp8 DoubleRow moving
                                       layout for the y projection
    y          = (Wp8.T @ outT8)/4096 + (x + bp)
                                       2 DoubleRow MMs per 128-row
                                       group; ScalarE+GpSimd (m<2) or a
                                       fused DVE affine_then_add (m>=2)
                                       do the rescale + bf16 residual.

All HBM traffic is chunk-major so each chunk moves with ONE descriptor-
friendly >=256KB dma_start per stream (inputs x8/xbf, half-chunk y
stores); weights load as single whole-tensor transfers. The host packs
the inputs into these layouts and unscrambles the chunk-major output.
Chunk-0's q projection runs first and doubles as the PE HAM warm-up.
"""

import numpy as np

import concourse.bass as bass
import concourse.mybir as mybir
import concourse.tile as tile
from concourse import bacc
from concourse.bass_utils import run_bass_kernel_spmd

F32 = mybir.dt.float32
BF16 = mybir.dt.bfloat16
FP8 = mybir.dt.float8e4
DR = mybir.MatmulPerfMode.DoubleRow

B, C, N = 8, 512, 4096
HW = 64
K, MEM, H, D = 64, 512, 8, 64
NW = 512                # columns of N processed per chunk
NCH = N // NW           # 8 chunks
CCH = C // 128          # 4 chunks of channels/partitions
N_CORES = 8
SQ = 64.0               # host pre-scale on Wq (folded into exp scale)
SO = 64.0               # on-chip scale on outT (via blockones=1/64)
SP = 64.0               # host pre-scale on Wp
YSCALE = 1.0 / (SO * SP)


def build_bass():
    nc = bacc.Bacc("TRN2", target_bir_lowering=False, debug=False)

    # chunk-major input/output layouts; one dma_start per chunk per stream
    xbbf = nc.dram_tensor("xbbf", [NCH * 128, CCH * NW], BF16,
                          kind="ExternalInput")
    xq = nc.dram_tensor("xq", [NCH * 128, 4 * NW], FP8, kind="ExternalInput")
    ftT = nc.dram_tensor("ftT", [128, CCH * K], BF16, kind="ExternalInput")
    wq8 = nc.dram_tensor("wq8", [128, 4 * C], FP8, kind="ExternalInput")
    wp8 = nc.dram_tensor("wp8", [128, 4 * C], FP8, kind="ExternalInput")
    wkT = nc.dram_tensor("wkT", [128, CCH * C], BF16, kind="ExternalInput")
    wvT = nc.dram_tensor("wvT", [128, CCH * C], BF16, kind="ExternalInput")
    bones = nc.dram_tensor("bones", [128, 128], BF16, kind="ExternalInput")
    yb = nc.dram_tensor("yb", [NCH * 128, CCH * NW], F32,
                        kind="ExternalOutput")

    with tile.TileContext(nc) as tc:
        _body(tc, xbbf, xq, ftT, wq8, wp8, wkT, wvT, bones, yb)
    nc.compile()
    return nc


def _body(tc, xbbf, xq, ftT, wq8, wp8, wkT, wvT, bones, yb):
    nc = tc.nc
    Exp = mybir.ActivationFunctionType.Exp

    with (
        tc.tile_pool(name="const", bufs=1) as const,
        tc.tile_pool(name="xbf", bufs=4) as xbfp,
        tc.tile_pool(name="xq8", bufs=3) as xqp,
        tc.tile_pool(name="qt", bufs=2) as qtp,
        tc.tile_pool(name="expt", bufs=3) as expp,
        tc.tile_pool(name="rcp", bufs=2) as rcp,
        tc.tile_pool(name="ycorr", bufs=2) as ycp,
        tc.tile_pool(name="out8", bufs=2) as outp,
        tc.tile_pool(name="yout", bufs=2) as yop,
        tc.tile_pool(name="ps_qy", bufs=2, space="PSUM") as ps_qy,
        tc.tile_pool(name="ps_s", bufs=2, space="PSUM") as ps_s,
        tc.tile_pool(name="ps_sb", bufs=2, space="PSUM") as ps_sb,
        tc.tile_pool(name="ps_o", bufs=2, space="PSUM") as ps_o,
    ):
        # ---- input loaders; x8 on the critical path, xbf deferred ----------
        hist = {}

        def load_x(c):
            st = {"i": c}
            t = xqp.tile([128, 2, 2, NW], FP8, name="x8_t", tag="x8")
            nc.sync.dma_start(out=t[:], in_=xq[128 * c:128 * (c + 1), :])
            st["x8"] = t
            st["qT"] = [None] * CCH
            st["expT"] = [None] * CCH
            st["rb"] = [None] * CCH
            return st

        def load_xbf(st):
            st["xbf"] = xbfp.tile([128, CCH, NW], BF16, name="xbf_t", tag="xbf")
            c = st["i"]
            nc.sync.dma_start(out=st["xbf"][:],
                              in_=xbbf[128 * c:128 * (c + 1), :])

        # DMA issue order = priority order: chunk-0 q inputs first, then the
        # k/v-projection weights, then everything else.
        hist[0] = load_x(0)
        wq_t = const.tile([128, 2, 2, C], FP8, tag="wq8")
        nc.sync.dma_start(out=wq_t[:], in_=wq8[:, :])
        ftT_t = const.tile([128, CCH, K], BF16, tag="ftT")
        nc.sync.dma_start(out=ftT_t[:], in_=ftT[:, :])
        wk_t = const.tile([128, CCH, C], BF16, tag="wkT")
        nc.sync.dma_start(out=wk_t[:], in_=wkT[:, :])
        hist[1] = load_x(1)
        wv_t = const.tile([128, CCH, C], BF16, tag="wvT")
        nc.sync.dma_start(out=wv_t[:], in_=wvT[:, :])
        bones_sb = const.tile([128, 128], BF16, tag="bones")
        nc.sync.dma_start(out=bones_sb[:], in_=bones[:, :])

        # ---- pipeline stage helpers ----------------------------------------
        kbd = []
        vbd = []

        def q_group(st, m):
            pq = ps_qy.tile([128, NW], F32, name="pq", tag="qy")
            for cb in range(2):
                nc.tensor.matmul(
                    pq[:],
                    lhsT=wq_t[:, cb, :, 128 * m:128 * (m + 1)],
                    rhs=st["x8"][:, cb, :, :],
                    start=(cb == 0),
                    stop=(cb == 1),
                    perf_mode=DR,
                )
            t = qtp.tile([128, NW], BF16, name="qT_t", tag=f"q{m}")
            nc.scalar.copy(t[:], pq[:])
            st["qT"][m] = t

        def s_stage(st, j):
            ps = ps_s.tile([128, NW], F32, name="ps", tag="ps")
            nc.tensor.matmul(ps[:], lhsT=kbd[j][:], rhs=st["qT"][j][:],
                             start=True, stop=True)
            t = expp.tile([128, NW], BF16, name="expT_t", tag=f"e{j}")
            nc.scalar.activation(t[:], ps[:], Exp, bias=0.0,
                                 scale=0.125 / SQ)
            st["expT"][j] = t

        def sb_stage(st, j):
            pb = ps_sb.tile([128, NW], F32, name="pb", tag="pb")
            nc.tensor.matmul(pb[:], lhsT=bones_sb[:], rhs=st["expT"][j][:],
                             start=True, stop=True)
            t = rcp.tile([128, NW], F32, name="rb_t", tag=f"r{j}")
            nc.vector.reciprocal_approx_fast(t[:], pb[:])
            st["rb"][j] = t

        def out_stage(st, j):
            if j == 0:
                st["o8"] = [
                    outp.tile([128, 2, NW], FP8, name="o8_t", tag=f"o8{cb}")
                    for cb in range(2)
                ]
                st["yo"] = yop.tile([128, CCH, NW], F32, name="yo_t", tag="yo")
            po = ps_o.tile([128, NW], F32, name="po", tag="po")
            nc.tensor.matmul(po[:], lhsT=vbd[j][:], rhs=st["expT"][j][:],
                             start=True, stop=True)
            with nc.allow_low_precision(reason="fp8 attention out tile"):
                nc.vector.tensor_mul(st["o8"][j // 2][:, j % 2, :],
                                     po[:], st["rb"][j][:])

        def y_group(st, m):
            py = ps_qy.tile([128, NW], F32, name="py", tag="qy")
            for cb in range(2):
                nc.tensor.matmul(
                    py[:],
                    lhsT=wp_t[:, cb, :, 128 * m:128 * (m + 1)],
                    rhs=st["o8"][cb][:],
                    start=(cb == 0),
                    stop=(cb == 1),
                    perf_mode=DR,
                )
            if m < 2:
                yc = ycp.tile([128, NW], BF16, name="yc_t", tag=f"yc{m}")
                with nc.allow_low_precision(reason="bf16 projection tail"):
                    nc.scalar.activation(
                        yc[:], py[:], mybir.ActivationFunctionType.Copy,
                        bias=0.0, scale=YSCALE,
                    )
                nc.gpsimd.tensor_add(st["yo"][:, m, :], yc[:],
                                     st["xbf"][:, m, :])
            else:
                nc.vector.affine_then_add(
                    st["yo"][:, m, :], py[:], st["xbf"][:, m, :],
                    scale=YSCALE, bias=0.0,
                )
            c = st["i"]
            if c == NCH - 1:
                nc.sync.dma_start(
                    out=yb[128 * c:128 * (c + 1), NW * m:NW * (m + 1)],
                    in_=st["yo"][:, m, :],
                )
            elif m % 2 == 1:
                nc.sync.dma_start(
                    out=yb[128 * c:128 * (c + 1), NW * (m - 1):NW * (m + 1)],
                    in_=st["yo"][:, m - 1:m + 1, :],
                )

        # ---- chunk-0 q projection first (it doubles as PE warm-up) ---------
        for m0 in range(CCH):
            q_group(hist[0], m0)
        wp_t = const.tile([128, 2, 2, C], FP8, tag="wp8")
        nc.sync.dma_start(out=wp_t[:], in_=wp8[:, :])
        load_xbf(hist[0])

        # ---- kT = Wk @ Ft^T, packed block-diagonal per head pair -----------
        for cj in range(CCH):
            t = const.tile([128, 128], BF16, tag=f"kbd{cj}")
            nc.vector.memset(t[:], 0.0)
            kbd.append(t)
        for cj in range(CCH):
            pk = ps_s.tile([128, NW], F32, tag="ps")
            for mk in range(CCH):
                nc.tensor.matmul(
                    pk[:, :K],
                    lhsT=wk_t[:, mk, 128 * cj:128 * (cj + 1)],
                    rhs=ftT_t[:, mk, :],
                    start=(mk == 0),
                    stop=(mk == CCH - 1),
                )
            nc.vector.tensor_copy(kbd[cj][0:64, 0:64], pk[0:64, :K])
            nc.vector.tensor_copy(kbd[cj][64:128, 64:128], pk[64:128, :K])

        # ---- v = Ft @ Wv^T [K, C], duplicated then packed block-diag -------
        v_dup = const.tile([128, C], BF16, tag="vdup")
        pv = ps_o.tile([128, NW], F32, tag="po")
        for mk in range(CCH):
            nc.tensor.matmul(
                pv[0:64, :],
                lhsT=ftT_t[:, mk, :],
                rhs=wv_t[:, mk, :],
                start=(mk == 0),
                stop=(mk == CCH - 1),
            )
        nc.vector.tensor_copy(v_dup[0:64, :], pv[0:64, :])
        nc.sync.dma_start(out=v_dup[64:128, :], in_=v_dup[0:64, :])
        for cj in range(CCH):
            t = const.tile([128, 128], BF16, tag=f"vbd{cj}")
            nc.vector.memset(t[:], 0.0)
            vbd.append(t)
        for cj in range(CCH):
            nc.vector.tensor_copy(vbd[cj][0:64, 0:64],
                                  v_dup[0:64, 128 * cj:128 * cj + 64])
            nc.vector.tensor_copy(vbd[cj][64:128, 64:128],
                                  v_dup[64:128, 128 * cj + 64:128 * cj + 128])

        # ---- main loop: 4-stream round-robin software pipeline -------------
        # iteration t runs: q(t), scores(t-1), sums+out(t-2), y(t-3); adjacent
        # PE groups always come from different streams, so every semaphore
        # wait is covered by independent matmul work and the PE never idles
        # long enough to drop the HAM clock.
        for t in range(1, NCH + 3):
            if t + 1 < NCH:
                hist[t + 1] = load_x(t + 1)
            if 1 <= t - 1 < NCH:
                load_xbf(hist[t - 1])
            qs = hist.get(t) if t < NCH else None
            ss = hist.get(t - 1)
            bo = hist.get(t - 2)
            ys = hist.get(t - 3)
            for r in range(CCH):
                if qs is not None:
                    q_group(qs, r)
                if ss is not None:
                    s_stage(ss, r)
                if bo is not None:
                    sb_stage(bo, r)
                    out_stage(bo, r)
                if ys is not None:
                    y_group(ys, r)
            if ys is not None:
                del hist[t - 3]


_NC_CACHE = None
LAST_RESULTS = None


def kernel(x, Ft, Wq, Wk, Wv, Wp, bp):
    global _NC_CACHE, LAST_RESULTS
    import ml_dtypes

    bf16 = ml_dtypes.bfloat16
    e4 = ml_dtypes.float8_e4m3

    def toe4(a):
        return np.clip(a, -240.0, 240.0).astype(e4)

    x = np.ascontiguousarray(np.asarray(x, dtype=np.float32))
    Ft = np.asarray(Ft, dtype=np.float32)

    # fp8 weights, one [128, cb, i, o] block: c = cb*256 + i*128 + p
    def pack_dr(WT_scaled):
        a = WT_scaled.reshape(2, 2, 128, C).transpose(2, 0, 1, 3)
        return np.ascontiguousarray(a.reshape(128, 4 * C))

    wq8 = toe4(pack_dr(np.asarray(Wq, dtype=np.float32).T * SQ))
    wp8 = toe4(pack_dr(np.asarray(Wp, dtype=np.float32).T * SP))
    # bf16 weights [128, mk, c]: row p, slot mk holds W.T[mk*128+p, c]
    wkT = np.ascontiguousarray(
        np.asarray(Wk, dtype=np.float32).T.reshape(CCH, 128, C)
        .transpose(1, 0, 2).reshape(128, CCH * C)).astype(bf16)
    wvT = np.ascontiguousarray(
        np.asarray(Wv, dtype=np.float32).T.reshape(CCH, 128, C)
        .transpose(1, 0, 2).reshape(128, CCH * C)).astype(bf16)
    ftT = np.ascontiguousarray(
        Ft.transpose(0, 2, 1).reshape(B, CCH, 128, K)
        .transpose(0, 2, 1, 3).reshape(B, 128, CCH * K)).astype(bf16)

    bones = np.zeros((128, 128), dtype=np.float32)
    bones[0:64, 0:64] = 1.0 / SO
    bones[64:128, 64:128] = 1.0 / SO
    bones = bones.astype(bf16)

    xr = x.reshape(B, C, N)
    # residual stream, chunk-major [ch*128+p, j*NW+nw] with bp folded in
    xrbf = (xr + np.asarray(bp, dtype=np.float32).reshape(1, C, 1)).astype(bf16)
    xrbf = xrbf.reshape(B, CCH, 128, NCH, NW).transpose(0, 3, 2, 1, 4)
    xrbf = np.ascontiguousarray(xrbf.reshape(B, NCH * 128, CCH * NW))
    # x fp8 DoubleRow chunk-major layout [ch*128+p, ((cb*2+i)*NW)+nw]
    x8 = toe4(xr).reshape(B, 2, 2, 128, NCH, NW).transpose(0, 4, 3, 1, 2, 5)
    x8 = np.ascontiguousarray(x8.reshape(B, NCH * 128, 4 * NW))

    if _NC_CACHE is None:
        _NC_CACHE = build_bass()
    nc = _NC_CACHE

    in_maps = [
        {
            "xbbf": xrbf[b],
            "xq": x8[b],
            "ftT": ftT[b],
            "wq8": wq8,
            "wp8": wp8,
            "wkT": wkT,
            "wvT": wvT,
            "bones": bones,
        }
        for b in range(B)
    ]
    res = run_bass_kernel_spmd(nc, in_maps, core_ids=list(range(N_CORES)))
    LAST_RESULTS = res
    y = np.stack([res.results[b]["yb"] for b in range(B)])
    # y chunk-major [ch*128+p, m*NW+nw] -> [c = m*128+p, n = ch*NW+nw]
    y = y.reshape(B, NCH, 128, CCH, NW).transpose(0, 3, 2, 1, 4)
    return np.ascontiguousarray(y.reshape(B, C, HW, HW))


# revision 19
# speedup vs baseline: 1.1566x; 1.1566x over previous
"""Trainium2 Bass kernel for nn_FMG_6717328851807 (dense_transformer).

Reference computation (B=8, C=512, H=W=64, K=64, MEM=512, heads=8, d=64):
    q = Wq @ x            (1x1 conv)          -> [B,h,N,d], N = H*W = 4096
    k = Ft @ Wk.T, v = Ft @ Wv.T              -> [B,h,K,d]
    attn = softmax(q k^T / sqrt(d))           -> [B,h,N,K]
    out = attn @ v                            -> [B,h,N,d]
    y = x + Wp @ out + bp

Sharding: pure data-parallel over B - one batch element per NeuronCore,
no collectives. Within a core everything runs transposed (channels on
partitions, spatial N on the free dim) in NW=512-column chunks, and
every PE op is a full-width matmul:

    qT[C,N]    = Wq8.T @ x8            fp8e4m3 DoubleRow: 2 MMs of
                                       256-deep contraction per 128-row
                                       group (Wq pre-scaled by 64 on the
                                       host; the 1/64 folds into the
                                       softmax exp scale)
    kT, v      = bf16 setup matmuls, then repacked into BLOCK-DIAGONAL
                 [128,128] tiles per head pair so that
    scoresT    = kbd.T @ qT            one 128-wide MM per head pair
    expT       = exp(scoresT/(8*64))   ScalarE, bf16 out
    sums_bc    = blockones.T @ expT    one MM per pair computes the
                                       softmax denominator AND
                                       broadcasts it to all 64
                                       partitions of its head
                                       (blockones holds 1/64, so the
                                       reciprocal is pre-scaled for the
                                       fp8 out tile)
    rb         = recip_approx(sums_bc) DVE, fp32
    outT8      = po * rb               DVE multiply, written straight
                                       into the fp8 DoubleRow moving
                                       layout for the y projection
    y          = (Wp8.T @ outT8)/4096 + (x + bp)
                                       2 DoubleRow MMs per 128-row
                                       group; ScalarE+GpSimd (m<2) or a
                                       fused DVE affine_then_add (m>=2)
                                       do the rescale + bf16 residual.

All HBM traffic is chunk-major so each chunk moves with ONE descriptor-
friendly >=256KB dma_start per stream (inputs x8/xbf, half-chunk y
stores); weights load as single whole-tensor transfers. The host packs
the inputs into these layouts and unscrambles the chunk-major output.
Chunk-0's q projection runs first and doubles as the PE HAM warm-up.
"""

import numpy as np

import concourse.bass as bass
import concourse.mybir as mybir
import concourse.tile as tile
from concourse import bacc
from concourse.bass_utils import run_bass_kernel_spmd

F32 = mybir.dt.float32
BF16 = mybir.dt.bfloat16
FP8 = mybir.dt.float8e4
DR = mybir.MatmulPerfMode.DoubleRow

B, C, N = 8, 512, 4096
HW = 64
K, MEM, H, D = 64, 512, 8, 64
NW = 512                # columns of N processed per chunk
NCH = N // NW           # 8 chunks
CCH = C // 128          # 4 chunks of channels/partitions
N_CORES = 8
SQ = 64.0               # host pre-scale on Wq (folded into exp scale)
SO = 64.0               # on-chip scale on outT (via blockones=1/64)
SP = 64.0               # host pre-scale on Wp
YSCALE = 1.0 / (SO * SP)


def build_bass():
    nc = bacc.Bacc("TRN2", target_bir_lowering=False, debug=False)

    # chunk-major input/output layouts; one dma_start per chunk per stream
    xbbf = nc.dram_tensor("xbbf", [NCH * 128, CCH * NW], BF16,
                          kind="ExternalInput")
    xq = nc.dram_tensor("xq", [NCH * 128, 4 * NW], FP8, kind="ExternalInput")
    ftT = nc.dram_tensor("ftT", [128, CCH * K], BF16, kind="ExternalInput")
    wq8 = nc.dram_tensor("wq8", [128, 4 * C], FP8, kind="ExternalInput")
    wp8 = nc.dram_tensor("wp8", [128, 4 * C], FP8, kind="ExternalInput")
    wkT = nc.dram_tensor("wkT", [128, CCH * C], BF16, kind="ExternalInput")
    wvT = nc.dram_tensor("wvT", [128, CCH * C], BF16, kind="ExternalInput")
    bones = nc.dram_tensor("bones", [128, 128], BF16, kind="ExternalInput")
    yb = nc.dram_tensor("yb", [NCH * 128, CCH * NW], F32,
                        kind="ExternalOutput")

    with tile.TileContext(nc) as tc:
        _body(tc, xbbf, xq, ftT, wq8, wp8, wkT, wvT, bones, yb)
    nc.compile()
    return nc


def _body(tc, xbbf, xq, ftT, wq8, wp8, wkT, wvT, bones, yb):
    nc = tc.nc
    Exp = mybir.ActivationFunctionType.Exp

    with (
        tc.tile_pool(name="const", bufs=1) as const,
        tc.tile_pool(name="xbf", bufs=4) as xbfp,
        tc.tile_pool(name="xq8", bufs=3) as xqp,
        tc.tile_pool(name="qt", bufs=2) as qtp,
        tc.tile_pool(name="expt", bufs=3) as expp,
        tc.tile_pool(name="rcp", bufs=2) as rcp,
        tc.tile_pool(name="ycorr", bufs=2) as ycp,
        tc.tile_pool(name="out8", bufs=2) as outp,
        tc.tile_pool(name="yout", bufs=2) as yop,
        tc.tile_pool(name="ps_qy", bufs=2, space="PSUM") as ps_qy,
        tc.tile_pool(name="ps_s", bufs=2, space="PSUM") as ps_s,
        tc.tile_pool(name="ps_sb", bufs=2, space="PSUM") as ps_sb,
        tc.tile_pool(name="ps_o", bufs=2, space="PSUM") as ps_o,
    ):
        # ---- input loaders; x8 on the critical path, xbf deferred ----------
        hist = {}

        def load_x(c):
            st = {"i": c}
            t = xqp.tile([128, 2, 2, NW], FP8, name="x8_t", tag="x8")
            nc.sync.dma_start(out=t[:], in_=xq[128 * c:128 * (c + 1), :])
            st["x8"] = t
            st["qT"] = [None] * CCH
            st["expT"] = [None] * CCH
            st["rb"] = [None] * CCH
            return st

        def load_xbf(st):
            st["xbf"] = xbfp.tile([128, CCH, NW], BF16, name="xbf_t", tag="xbf")
            c = st["i"]
            nc.sync.dma_start(out=st["xbf"][:],
                              in_=xbbf[128 * c:128 * (c + 1), :])

        # DMA issue order = priority order: chunk-0 q inputs first, then the
        # k/v-projection weights, then everything else.
        hist[0] = load_x(0)
        wq_t = const.tile([128, 2, 2, C], FP8, tag="wq8")
        nc.sync.dma_start(out=wq_t[:], in_=wq8[:, :])
        ftT_t = const.tile([128, CCH, K], BF16, tag="ftT")
        nc.sync.dma_start(out=ftT_t[:], in_=ftT[:, :])
        wk_t = const.tile([128, CCH, C], BF16, tag="wkT")
        nc.sync.dma_start(out=wk_t[:], in_=wkT[:, :])
        hist[1] = load_x(1)
        wv_t = const.tile([128, CCH, C], BF16, tag="wvT")
        nc.sync.dma_start(out=wv_t[:], in_=wvT[:, :])
        bones_sb = const.tile([128, 128], BF16, tag="bones")
        nc.sync.dma_start(out=bones_sb[:], in_=bones[:, :])
        wp_t = const.tile([128, 2, 2, C], FP8, tag="wp8")
        nc.sync.dma_start(out=wp_t[:], in_=wp8[:, :])
        load_xbf(hist[0])

        # ---- pipeline stage helpers ----------------------------------------
        kbd = []
        vbd = []

        def q_group(st, m):
            pq = ps_qy.tile([128, NW], F32, name="pq", tag="qy")
            for cb in range(2):
                nc.tensor.matmul(
                    pq[:],
                    lhsT=wq_t[:, cb, :, 128 * m:128 * (m + 1)],
                    rhs=st["x8"][:, cb, :, :],
                    start=(cb == 0),
                    stop=(cb == 1),
                    perf_mode=DR,
                )
            t = qtp.tile([128, NW], BF16, name="qT_t", tag=f"q{m}")
            nc.scalar.copy(t[:], pq[:])
            st["qT"][m] = t

        def s_stage(st, j):
            ps = ps_s.tile([128, NW], F32, name="ps", tag="ps")
            nc.tensor.matmul(ps[:], lhsT=kbd[j][:], rhs=st["qT"][j][:],
                             start=True, stop=True)
            t = expp.tile([128, NW], BF16, name="expT_t", tag=f"e{j}")
            nc.scalar.activation(t[:], ps[:], Exp, bias=0.0,
                                 scale=0.125 / SQ)
            st["expT"][j] = t

        def sb_stage(st, j):
            pb = ps_sb.tile([128, NW], F32, name="pb", tag="pb")
            nc.tensor.matmul(pb[:], lhsT=bones_sb[:], rhs=st["expT"][j][:],
                             start=True, stop=True)
            t = rcp.tile([128, NW], F32, name="rb_t", tag=f"r{j}")
            nc.vector.reciprocal_approx_fast(t[:], pb[:])
            st["rb"][j] = t

        def out_stage(st, j):
            if j == 0:
                st["o8"] = [
                    outp.tile([128, 2, NW], FP8, name="o8_t", tag=f"o8{cb}")
                    for cb in range(2)
                ]
                st["yo"] = yop.tile([128, CCH, NW], F32, name="yo_t", tag="yo")
            po = ps_o.tile([128, NW], F32, name="po", tag="po")
            nc.tensor.matmul(po[:], lhsT=vbd[j][:], rhs=st["expT"][j][:],
                             start=True, stop=True)
            with nc.allow_low_precision(reason="fp8 attention out tile"):
                nc.vector.tensor_mul(st["o8"][j // 2][:, j % 2, :],
                                     po[:], st["rb"][j][:])

        def y_group(st, m):
            py = ps_qy.tile([128, NW], F32, name="py", tag="qy")
            for cb in range(2):
                nc.tensor.matmul(
                    py[:],
                    lhsT=wp_t[:, cb, :, 128 * m:128 * (m + 1)],
                    rhs=st["o8"][cb][:],
                    start=(cb == 0),
                    stop=(cb == 1),
                    perf_mode=DR,
                )
            if m < 2:
                yc = ycp.tile([128, NW], BF16, name="yc_t", tag=f"yc{m}")
                with nc.allow_low_precision(reason="bf16 projection tail"):
                    nc.scalar.activation(
                        yc[:], py[:], mybir.ActivationFunctionType.Copy,
                        bias=0.0, scale=YSCALE,
                    )
                nc.gpsimd.tensor_add(st["yo"][:, m, :], yc[:],
                                     st["xbf"][:, m, :])
            else:
                nc.vector.affine_then_add(
                    st["yo"][:, m, :], py[:], st["xbf"][:, m, :],
                    scale=YSCALE, bias=0.0,
                )
            if m % 2 == 1:
                c = st["i"]
                nc.sync.dma_start(
                    out=yb[128 * c:128 * (c + 1), NW * (m - 1):NW * (m + 1)],
                    in_=st["yo"][:, m - 1:m + 1, :],
                )

        # ---- chunk-0 q projection first (it doubles as PE warm-up) ---------
        for m0 in range(CCH):
            q_group(hist[0], m0)

        # ---- kT = Wk @ Ft^T, packed block-diagonal per head pair -----------
        for cj in range(CCH):
            t = const.tile([128, 128], BF16, tag=f"kbd{cj}")
            nc.vector.memset(t[:], 0.0)
            kbd.append(t)
        for cj in range(CCH):
            pk = ps_s.tile([128, NW], F32, tag="ps")
            for mk in range(CCH):
                nc.tensor.matmul(
                    pk[:, :K],
                    lhsT=wk_t[:, mk, 128 * cj:128 * (cj + 1)],
                    rhs=ftT_t[:, mk, :],
                    start=(mk == 0),
                    stop=(mk == CCH - 1),
                )
            nc.vector.tensor_copy(kbd[cj][0:64, 0:64], pk[0:64, :K])
            nc.vector.tensor_copy(kbd[cj][64:128, 64:128], pk[64:128, :K])

        # ---- v = Ft @ Wv^T [K, C], duplicated then packed block-diag -------
        v_dup = const.tile([128, C], BF16, tag="vdup")
        pv = ps_o.tile([128, NW], F32, tag="po")
        for mk in range(CCH):
            nc.tensor.matmul(
                pv[0:64, :],
                lhsT=ftT_t[:, mk, :],
                rhs=wv_t[:, mk, :],
                start=(mk == 0),
                stop=(mk == CCH - 1),
            )
        nc.vector.tensor_copy(v_dup[0:64, :], pv[0:64, :])
        nc.sync.dma_start(out=v_dup[64:128, :], in_=v_dup[0:64, :])
        for cj in range(CCH):
            t = const.tile([128, 128], BF16, tag=f"vbd{cj}")
            nc.vector.memset(t[:], 0.0)
            vbd.append(t)
        for cj in range(CCH):
            nc.vector.tensor_copy(vbd[cj][0:64, 0:64],
                                  v_dup[0:64, 128 * cj:128 * cj + 64])
            nc.vector.tensor_copy(vbd[cj][64:128, 64:128],
                                  v_dup[64:128, 128 * cj + 64:128 * cj + 128])

        # ---- main loop: 4-stream round-robin software pipeline -------------
        # iteration t runs: q(t), scores(t-1), sums+out(t-2), y(t-3); adjacent
        # PE groups always come from different streams, so every semaphore
        # wait is covered by independent matmul work and the PE never idles
        # long enough to drop the HAM clock.
        for t in range(1, NCH + 3):
            if t + 1 < NCH:
                hist[t + 1] = load_x(t + 1)
            if 1 <= t - 1 < NCH:
                load_xbf(hist[t - 1])
            qs = hist.get(t) if t < NCH else None
            ss = hist.get(t - 1)
            bo = hist.get(t - 2)
            ys = hist.get(t - 3)
            for r in range(CCH):
                if qs is not None:
                    q_group(qs, r)
                if ss is not None:
                    s_stage(ss, r)
                if bo is not None:
                    sb_stage(bo, r)
                    out_stage(bo, r)
                if ys is not None:
                    y_group(ys, r)
            if ys is not None:
                del hist[t - 3]


_NC_CACHE = None
LAST_RESULTS = None


def kernel(x, Ft, Wq, Wk, Wv, Wp, bp):
    global _NC_CACHE, LAST_RESULTS
    import ml_dtypes

    bf16 = ml_dtypes.bfloat16
    e4 = ml_dtypes.float8_e4m3

    def toe4(a):
        return np.clip(a, -240.0, 240.0).astype(e4)

    x = np.ascontiguousarray(np.asarray(x, dtype=np.float32))
    Ft = np.asarray(Ft, dtype=np.float32)

    # fp8 weights, one [128, cb, i, o] block: c = cb*256 + i*128 + p
    def pack_dr(WT_scaled):
        a = WT_scaled.reshape(2, 2, 128, C).transpose(2, 0, 1, 3)
        return np.ascontiguousarray(a.reshape(128, 4 * C))

    wq8 = toe4(pack_dr(np.asarray(Wq, dtype=np.float32).T * SQ))
    wp8 = toe4(pack_dr(np.asarray(Wp, dtype=np.float32).T * SP))
    # bf16 weights [128, mk, c]: row p, slot mk holds W.T[mk*128+p, c]
    wkT = np.ascontiguousarray(
        np.asarray(Wk, dtype=np.float32).T.reshape(CCH, 128, C)
        .transpose(1, 0, 2).reshape(128, CCH * C)).astype(bf16)
    wvT = np.ascontiguousarray(
        np.asarray(Wv, dtype=np.float32).T.reshape(CCH, 128, C)
        .transpose(1, 0, 2).reshape(128, CCH * C)).astype(bf16)
    ftT = np.ascontiguousarray(
        Ft.transpose(0, 2, 1).reshape(B, CCH, 128, K)
        .transpose(0, 2, 1, 3).reshape(B, 128, CCH * K)).astype(bf16)

    bones = np.zeros((128, 128), dtype=np.float32)
    bones[0:64, 0:64] = 1.0 / SO
    bones[64:128, 64:128] = 1.0 / SO
    bones = bones.astype(bf16)

    xr = x.reshape(B, C, N)
    # residual stream, chunk-major [ch*128+p, j*NW+nw] with bp folded in
    xrbf = (xr + np.asarray(bp, dtype=np.float32).reshape(1, C, 1)).astype(bf16)
    xrbf = xrbf.reshape(B, CCH, 128, NCH, NW).transpose(0, 3, 2, 1, 4)
    xrbf = np.ascontiguousarray(xrbf.reshape(B, NCH * 128, CCH * NW))
    # x fp8 DoubleRow chunk-major layout [ch*128+p, ((cb*2+i)*NW)+nw]
    x8 = toe4(xr).reshape(B, 2, 2, 128, NCH, NW).transpose(0, 4, 3, 1, 2, 5)
    x8 = np.ascontiguousarray(x8.reshape(B, NCH * 128, 4 * NW))

    if _NC_CACHE is None:
        _NC_CACHE = build_bass()
    nc = _NC_CACHE

    in_maps = [
        {
            "xbbf": xrbf[b],
            "xq": x8[b],
            "ftT": ftT[b],
            "wq8": wq8,
            "wp8": wp8,
            "wkT": wkT,
            "wvT": wvT,
            "bones": bones,
        }
        for b in range(B)
    ]
    res = run_bass_kernel_spmd(nc, in_maps, core_ids=list(range(N_CORES)))
    LAST_RESULTS = res
    y = np.stack([res.results[b]["yb"] for b in range(B)])
    # y chunk-major [ch*128+p, m*NW+nw] -> [c = m*128+p, n = ch*NW+nw]
    y = y.reshape(B, NCH, 128, CCH, NW).transpose(0, 3, 2, 1, 4)
    return np.ascontiguousarray(y.reshape(B, C, HW, HW))


# revision 20
# speedup vs baseline: 1.1593x; 1.0023x over previous
"""Trainium2 Bass kernel for nn_FMG_6717328851807 (dense_transformer).

Reference computation (B=8, C=512, H=W=64, K=64, MEM=512, heads=8, d=64):
    q = Wq @ x            (1x1 conv)          -> [B,h,N,d], N = H*W = 4096
    k = Ft @ Wk.T, v = Ft @ Wv.T              -> [B,h,K,d]
    attn = softmax(q k^T / sqrt(d))           -> [B,h,N,K]
    out = attn @ v                            -> [B,h,N,d]
    y = x + Wp @ out + bp

Sharding: pure data-parallel over B - one batch element per NeuronCore,
no collectives. Within a core everything runs transposed (channels on
partitions, spatial N on the free dim) in NW=512-column chunks, and
every PE op is a full-width matmul:

    qT[C,N]    = Wq8.T @ x8            fp8e4m3 DoubleRow: 2 MMs of
                                       256-deep contraction per 128-row
                                       group (Wq pre-scaled by 64 on the
                                       host; the 1/64 folds into the
                                       softmax exp scale)
    kT, v      = bf16 setup matmuls, then repacked into BLOCK-DIAGONAL
                 [128,128] tiles per head pair so that
    scoresT    = kbd.T @ qT            one 128-wide MM per head pair
    expT       = exp(scoresT/(8*64))   ScalarE, bf16 out
    sums_bc    = blockones.T @ expT    one MM per pair computes the
                                       softmax denominator AND
                                       broadcasts it to all 64
                                       partitions of its head
                                       (blockones holds 1/64, so the
                                       reciprocal is pre-scaled for the
                                       fp8 out tile)
    rb         = recip_approx(sums_bc) DVE, fp32
    outT8      = po * rb               DVE multiply, written straight
                                       into the fp8 DoubleRow moving
                                       layout for the y projection
    y          = (Wp8.T @ outT8)/4096 + (x + bp)
                                       2 DoubleRow MMs per 128-row
                                       group; ScalarE+GpSimd (m<2) or a
                                       fused DVE affine_then_add (m>=2)
                                       do the rescale + bf16 residual.

All HBM traffic is chunk-major so each chunk moves with ONE descriptor-
friendly >=256KB dma_start per stream (inputs x8/xbf, half-chunk y
stores); weights load as single whole-tensor transfers. The host packs
the inputs into these layouts and unscrambles the chunk-major output.
Chunk-0's q projection runs first and doubles as the PE HAM warm-up.
"""

import numpy as np

import concourse.bass as bass
import concourse.mybir as mybir
import concourse.tile as tile
from concourse import bacc
from concourse.bass_utils import run_bass_kernel_spmd

F32 = mybir.dt.float32
BF16 = mybir.dt.bfloat16
FP8 = mybir.dt.float8e4
DR = mybir.MatmulPerfMode.DoubleRow

B, C, N = 8, 512, 4096
HW = 64
K, MEM, H, D = 64, 512, 8, 64
NW = 512                # columns of N processed per chunk
NCH = N // NW           # 8 chunks
CCH = C // 128          # 4 chunks of channels/partitions
N_CORES = 8
SQ = 64.0               # host pre-scale on Wq (folded into exp scale)
SO = 64.0               # on-chip scale on outT (via blockones=1/64)
SP = 64.0               # host pre-scale on Wp
YSCALE = 1.0 / (SO * SP)


def build_bass():
    nc = bacc.Bacc("TRN2", target_bir_lowering=False, debug=False)

    # chunk-major input/output layouts; one dma_start per chunk per stream
    xbbf = nc.dram_tensor("xbbf", [NCH * 128, CCH * NW], BF16,
                          kind="ExternalInput")
    xq = nc.dram_tensor("xq", [NCH * 128, 4 * NW], FP8, kind="ExternalInput")
    ftT = nc.dram_tensor("ftT", [128, CCH * K], BF16, kind="ExternalInput")
    wq8 = nc.dram_tensor("wq8", [128, 4 * C], FP8, kind="ExternalInput")
    wp8 = nc.dram_tensor("wp8", [128, 4 * C], FP8, kind="ExternalInput")
    wkT = nc.dram_tensor("wkT", [128, CCH * C], BF16, kind="ExternalInput")
    wvT = nc.dram_tensor("wvT", [128, CCH * C], BF16, kind="ExternalInput")
    bones = nc.dram_tensor("bones", [128, 128], BF16, kind="ExternalInput")
    yb = nc.dram_tensor("yb", [NCH * 128, CCH * NW], F32,
                        kind="ExternalOutput")

    with tile.TileContext(nc) as tc:
        _body(tc, xbbf, xq, ftT, wq8, wp8, wkT, wvT, bones, yb)
    nc.compile()
    return nc


def _body(tc, xbbf, xq, ftT, wq8, wp8, wkT, wvT, bones, yb):
    nc = tc.nc
    Exp = mybir.ActivationFunctionType.Exp

    with (
        tc.tile_pool(name="const", bufs=1) as const,
        tc.tile_pool(name="xbf", bufs=4) as xbfp,
        tc.tile_pool(name="xq8", bufs=3) as xqp,
        tc.tile_pool(name="qt", bufs=2) as qtp,
        tc.tile_pool(name="expt", bufs=3) as expp,
        tc.tile_pool(name="rcp", bufs=2) as rcp,
        tc.tile_pool(name="ycorr", bufs=2) as ycp,
        tc.tile_pool(name="out8", bufs=2) as outp,
        tc.tile_pool(name="yout", bufs=2) as yop,
        tc.tile_pool(name="ps_qy", bufs=2, space="PSUM") as ps_qy,
        tc.tile_pool(name="ps_s", bufs=2, space="PSUM") as ps_s,
        tc.tile_pool(name="ps_sb", bufs=2, space="PSUM") as ps_sb,
        tc.tile_pool(name="ps_o", bufs=2, space="PSUM") as ps_o,
    ):
        # ---- input loaders; x8 on the critical path, xbf deferred ----------
        hist = {}

        def load_x(c):
            st = {"i": c}
            t = xqp.tile([128, 2, 2, NW], FP8, name="x8_t", tag="x8")
            nc.sync.dma_start(out=t[:], in_=xq[128 * c:128 * (c + 1), :])
            st["x8"] = t
            st["qT"] = [None] * CCH
            st["expT"] = [None] * CCH
            st["rb"] = [None] * CCH
            return st

        def load_xbf(st):
            st["xbf"] = xbfp.tile([128, CCH, NW], BF16, name="xbf_t", tag="xbf")
            c = st["i"]
            nc.sync.dma_start(out=st["xbf"][:],
                              in_=xbbf[128 * c:128 * (c + 1), :])

        # DMA issue order = priority order: chunk-0 q inputs first, then the
        # k/v-projection weights, then everything else.
        hist[0] = load_x(0)
        wq_t = const.tile([128, 2, 2, C], FP8, tag="wq8")
        nc.sync.dma_start(out=wq_t[:], in_=wq8[:, :])
        ftT_t = const.tile([128, CCH, K], BF16, tag="ftT")
        nc.sync.dma_start(out=ftT_t[:], in_=ftT[:, :])
        wk_t = const.tile([128, CCH, C], BF16, tag="wkT")
        nc.sync.dma_start(out=wk_t[:], in_=wkT[:, :])
        hist[1] = load_x(1)
        wv_t = const.tile([128, CCH, C], BF16, tag="wvT")
        nc.sync.dma_start(out=wv_t[:], in_=wvT[:, :])
        bones_sb = const.tile([128, 128], BF16, tag="bones")
        nc.sync.dma_start(out=bones_sb[:], in_=bones[:, :])
        wp_t = const.tile([128, 2, 2, C], FP8, tag="wp8")
        nc.sync.dma_start(out=wp_t[:], in_=wp8[:, :])
        load_xbf(hist[0])

        # ---- pipeline stage helpers ----------------------------------------
        kbd = []
        vbd = []

        def q_group(st, m):
            pq = ps_qy.tile([128, NW], F32, name="pq", tag="qy")
            for cb in range(2):
                nc.tensor.matmul(
                    pq[:],
                    lhsT=wq_t[:, cb, :, 128 * m:128 * (m + 1)],
                    rhs=st["x8"][:, cb, :, :],
                    start=(cb == 0),
                    stop=(cb == 1),
                    perf_mode=DR,
                )
            t = qtp.tile([128, NW], BF16, name="qT_t", tag=f"q{m}")
            nc.scalar.copy(t[:], pq[:])
            st["qT"][m] = t

        def s_stage(st, j):
            ps = ps_s.tile([128, NW], F32, name="ps", tag="ps")
            nc.tensor.matmul(ps[:], lhsT=kbd[j][:], rhs=st["qT"][j][:],
                             start=True, stop=True)
            t = expp.tile([128, NW], BF16, name="expT_t", tag=f"e{j}")
            nc.scalar.activation(t[:], ps[:], Exp, bias=0.0,
                                 scale=0.125 / SQ)
            st["expT"][j] = t

        def sb_stage(st, j):
            pb = ps_sb.tile([128, NW], F32, name="pb", tag="pb")
            nc.tensor.matmul(pb[:], lhsT=bones_sb[:], rhs=st["expT"][j][:],
                             start=True, stop=True)
            t = rcp.tile([128, NW], F32, name="rb_t", tag=f"r{j}")
            nc.vector.reciprocal_approx_fast(t[:], pb[:])
            st["rb"][j] = t

        def out_stage(st, j):
            if j == 0:
                st["o8"] = [
                    outp.tile([128, 2, NW], FP8, name="o8_t", tag=f"o8{cb}")
                    for cb in range(2)
                ]
                st["yo"] = yop.tile([128, CCH, NW], F32, name="yo_t", tag="yo")
            po = ps_o.tile([128, NW], F32, name="po", tag="po")
            nc.tensor.matmul(po[:], lhsT=vbd[j][:], rhs=st["expT"][j][:],
                             start=True, stop=True)
            with nc.allow_low_precision(reason="fp8 attention out tile"):
                nc.vector.tensor_mul(st["o8"][j // 2][:, j % 2, :],
                                     po[:], st["rb"][j][:])

        def y_group(st, m):
            py = ps_qy.tile([128, NW], F32, name="py", tag="qy")
            for cb in range(2):
                nc.tensor.matmul(
                    py[:],
                    lhsT=wp_t[:, cb, :, 128 * m:128 * (m + 1)],
                    rhs=st["o8"][cb][:],
                    start=(cb == 0),
                    stop=(cb == 1),
                    perf_mode=DR,
                )
            if m < 2:
                yc = ycp.tile([128, NW], BF16, name="yc_t", tag=f"yc{m}")
                with nc.allow_low_precision(reason="bf16 projection tail"):
                    nc.scalar.activation(
                        yc[:], py[:], mybir.ActivationFunctionType.Copy,
                        bias=0.0, scale=YSCALE,
                    )
                nc.gpsimd.tensor_add(st["yo"][:, m, :], yc[:],
                                     st["xbf"][:, m, :])
            else:
                nc.vector.affine_then_add(
                    st["yo"][:, m, :], py[:], st["xbf"][:, m, :],
                    scale=YSCALE, bias=0.0,
                )
            if m % 2 == 1:
                c = st["i"]
                nc.sync.dma_start(
                    out=yb[128 * c:128 * (c + 1), NW * (m - 1):NW * (m + 1)],
                    in_=st["yo"][:, m - 1:m + 1, :],
                )

        # ---- chunk-0 q projection first (it doubles as PE warm-up) ---------
        for m0 in range(CCH):
            q_group(hist[0], m0)

        # ---- kT = Wk @ Ft^T, packed block-diagonal per head pair -----------
        for cj in range(CCH):
            t = const.tile([128, 128], BF16, tag=f"kbd{cj}")
            nc.vector.memset(t[:], 0.0)
            kbd.append(t)
        for cj in range(CCH):
            pk = ps_s.tile([128, NW], F32, tag="ps")
            for mk in range(CCH):
                nc.tensor.matmul(
                    pk[:, :K],
                    lhsT=wk_t[:, mk, 128 * cj:128 * (cj + 1)],
                    rhs=ftT_t[:, mk, :],
                    start=(mk == 0),
                    stop=(mk == CCH - 1),
                )
            nc.vector.tensor_copy(kbd[cj][0:64, 0:64], pk[0:64, :K])
            nc.vector.tensor_copy(kbd[cj][64:128, 64:128], pk[64:128, :K])

        # ---- v = Ft @ Wv^T [K, C], duplicated then packed block-diag -------
        v_dup = const.tile([128, C], BF16, tag="vdup")
        pv = ps_o.tile([128, NW], F32, tag="po")
        for mk in range(CCH):
            nc.tensor.matmul(
                pv[0:64, :],
                lhsT=ftT_t[:, mk, :],
                rhs=wv_t[:, mk, :],
                start=(mk == 0),
                stop=(mk == CCH - 1),
            )
        nc.vector.tensor_copy(v_dup[0:64, :], pv[0:64, :])
        nc.sync.dma_start(out=v_dup[64:128, :], in_=v_dup[0:64, :])
        for cj in range(CCH):
            t = const.tile([128, 128], BF16, tag=f"vbd{cj}")
            nc.vector.memset(t[:], 0.0)
            vbd.append(t)
        for cj in range(CCH):
            nc.vector.tensor_copy(vbd[cj][0:64, 0:64],
                                  v_dup[0:64, 128 * cj:128 * cj + 64])
            nc.vector.tensor_copy(vbd[cj][64:128, 64:128],
                                  v_dup[64:128, 128 * cj + 64:128 * cj + 128])

        # ---- main loop: 4-stream round-robin software pipeline -------------
        # iteration t runs: q(t), scores(t-1), sums+out(t-2), y(t-3); adjacent
        # PE groups always come from different streams, so every semaphore
        # wait is covered by independent matmul work and the PE never idles
        # long enough to drop the HAM clock.
        for t in range(1, NCH + 3):
            if t + 1 < NCH:
                hist[t + 1] = load_x(t + 1)
            if 1 <= t - 1 < NCH:
                load_xbf(hist[t - 1])
            qs = hist.get(t) if t < NCH else None
            ss = hist.get(t - 1)
            bo = hist.get(t - 2)
            ys = hist.get(t - 3)
            for r in range(CCH):
                if ys is not None:
                    y_group(ys, r)
                if qs is not None:
                    q_group(qs, r)
                if ss is not None:
                    s_stage(ss, r)
                if bo is not None:
                    sb_stage(bo, r)
                    out_stage(bo, r)
            if ys is not None:
                del hist[t - 3]


_NC_CACHE = None
LAST_RESULTS = None


def kernel(x, Ft, Wq, Wk, Wv, Wp, bp):
    global _NC_CACHE, LAST_RESULTS
    import ml_dtypes

    bf16 = ml_dtypes.bfloat16
    e4 = ml_dtypes.float8_e4m3

    def toe4(a):
        return np.clip(a, -240.0, 240.0).astype(e4)

    x = np.ascontiguousarray(np.asarray(x, dtype=np.float32))
    Ft = np.asarray(Ft, dtype=np.float32)

    # fp8 weights, one [128, cb, i, o] block: c = cb*256 + i*128 + p
    def pack_dr(WT_scaled):
        a = WT_scaled.reshape(2, 2, 128, C).transpose(2, 0, 1, 3)
        return np.ascontiguousarray(a.reshape(128, 4 * C))

    wq8 = toe4(pack_dr(np.asarray(Wq, dtype=np.float32).T * SQ))
    wp8 = toe4(pack_dr(np.asarray(Wp, dtype=np.float32).T * SP))
    # bf16 weights [128, mk, c]: row p, slot mk holds W.T[mk*128+p, c]
    wkT = np.ascontiguousarray(
        np.asarray(Wk, dtype=np.float32).T.reshape(CCH, 128, C)
        .transpose(1, 0, 2).reshape(128, CCH * C)).astype(bf16)
    wvT = np.ascontiguousarray(
        np.asarray(Wv, dtype=np.float32).T.reshape(CCH, 128, C)
        .transpose(1, 0, 2).reshape(128, CCH * C)).astype(bf16)
    ftT = np.ascontiguousarray(
        Ft.transpose(0, 2, 1).reshape(B, CCH, 128, K)
        .transpose(0, 2, 1, 3).reshape(B, 128, CCH * K)).astype(bf16)

    bones = np.zeros((128, 128), dtype=np.float32)
    bones[0:64, 0:64] = 1.0 / SO
    bones[64:128, 64:128] = 1.0 / SO
    bones = bones.astype(bf16)

    xr = x.reshape(B, C, N)
    # residual stream, chunk-major [ch*128+p, j*NW+nw] with bp folded in
    xrbf = (xr + np.asarray(bp, dtype=np.float32).reshape(1, C, 1)).astype(bf16)
    xrbf = xrbf.reshape(B, CCH, 128, NCH, NW).transpose(0, 3, 2, 1, 4)
    xrbf = np.ascontiguousarray(xrbf.reshape(B, NCH * 128, CCH * NW))
    # x fp8 DoubleRow chunk-major layout [ch*128+p, ((cb*2+i)*NW)+nw]
    x8 = toe4(xr).reshape(B, 2, 2, 128, NCH, NW).transpose(0, 4, 3, 1, 2, 5)
    x8 = np.ascontiguousarray(x8.reshape(B, NCH * 128, 4 * NW))

    if _NC_CACHE is None:
        _NC_CACHE = build_bass()
    nc = _NC_CACHE

    in_maps = [
        {
            "xbbf": xrbf[b],
            "xq": x8[b],
            "ftT": ftT[b],
            "wq8": wq8,
            "wp8": wp8,
            "wkT": wkT,
            "wvT": wvT,
            "bones": bones,
        }
        for b in range(B)
    ]
    res = run_bass_kernel_spmd(nc, in_maps, core_ids=list(range(N_CORES)))
    LAST_RESULTS = res
    y = np.stack([res.results[b]["yb"] for b in range(B)])
    # y chunk-major [ch*128+p, m*NW+nw] -> [c = m*128+p, n = ch*NW+nw]
    y = y.reshape(B, NCH, 128, CCH, NW).transpose(0, 3, 2, 1, 4)
    return np.ascontiguousarray(y.reshape(B, C, HW, HW))


# revision 22
# speedup vs baseline: 1.2524x; 1.0804x over previous
"""Trainium2 Bass kernel for nn_FMG_6717328851807 (dense_transformer).

Reference computation (B=8, C=512, H=W=64, K=64, MEM=512, heads=8, d=64):
    q = Wq @ x            (1x1 conv)          -> [B,h,N,d], N = H*W = 4096
    k = Ft @ Wk.T, v = Ft @ Wv.T              -> [B,h,K,d]
    attn = softmax(q k^T / sqrt(d))           -> [B,h,N,K]
    out = attn @ v                            -> [B,h,N,d]
    y = x + Wp @ out + bp

Sharding: pure data-parallel over B - one batch element per NeuronCore,
no collectives. Within a core everything runs transposed (channels on
partitions, spatial N on the free dim) in NW=512-column chunks, and
every PE op is a full-width matmul:

    qT[C,N]    = Wq8.T @ x8            fp8e4m3 DoubleRow: 2 MMs of
                                       256-deep contraction per 128-row
                                       group (Wq pre-scaled by 64 on the
                                       host; the 1/64 folds into the
                                       softmax exp scale)
    kT, v      = bf16 setup matmuls, then repacked into BLOCK-DIAGONAL
                 [128,128] tiles per head pair so that
    scoresT    = kbd.T @ qT            one 128-wide MM per head pair
    expT       = exp(scoresT/(8*64))   ScalarE, bf16 out
    sums_bc    = blockones.T @ expT    one MM per pair computes the
                                       softmax denominator AND
                                       broadcasts it to all 64
                                       partitions of its head
                                       (blockones holds 1/64, so the
                                       reciprocal is pre-scaled for the
                                       fp8 out tile)
    rb         = recip_approx(sums_bc) DVE, fp32
    outT8      = po * rb               DVE multiply, written straight
                                       into the fp8 DoubleRow moving
                                       layout for the y projection
    y          = (Wp8.T @ outT8)/4096 + (x + bp)
                                       2 DoubleRow MMs per 128-row
                                       group; ScalarE+GpSimd (m<2) or a
                                       fused DVE affine_then_add (m>=2)
                                       do the rescale + bf16 residual.

All HBM traffic is chunk-major so each chunk moves with ONE descriptor-
friendly >=256KB dma_start per stream (inputs x8/xbf, half-chunk y
stores); weights load as single whole-tensor transfers. The host packs
the inputs into these layouts and unscrambles the chunk-major output.
Chunk-0's q projection runs first and doubles as the PE HAM warm-up.
"""

import numpy as np

import concourse.bass as bass
import concourse.mybir as mybir
import concourse.tile as tile
from concourse import bacc
from concourse.bass_utils import run_bass_kernel_spmd

F32 = mybir.dt.float32
BF16 = mybir.dt.bfloat16
FP8 = mybir.dt.float8e4
DR = mybir.MatmulPerfMode.DoubleRow

B, C, N = 8, 512, 4096
HW = 64
K, MEM, H, D = 64, 512, 8, 64
NW = 512                # columns of N processed per chunk
NCH = N // NW           # 8 chunks
CCH = C // 128          # 4 chunks of channels/partitions
N_CORES = 8
SQ2 = 128.0             # on-chip scale on the fused score weights
SO = 64.0               # on-chip scale on outT (via blockones=1/64)
SP = 64.0               # host pre-scale on Wp
YSCALE = 1.0 / (SO * SP)


def build_bass():
    nc = bacc.Bacc("TRN2", target_bir_lowering=False, debug=False)

    # chunk-major input/output layouts; one dma_start per chunk per stream
    xbbf = nc.dram_tensor("xbbf", [NCH * 128, CCH * NW], BF16,
                          kind="ExternalInput")
    xq = nc.dram_tensor("xq", [NCH * 128, 4 * NW], FP8, kind="ExternalInput")
    ftT = nc.dram_tensor("ftT", [128, CCH * K], BF16, kind="ExternalInput")
    wqr = nc.dram_tensor("wqr", [128, CCH * C], BF16, kind="ExternalInput")
    wp8 = nc.dram_tensor("wp8", [128, 4 * C], FP8, kind="ExternalInput")
    wkT = nc.dram_tensor("wkT", [128, CCH * C], BF16, kind="ExternalInput")
    wvT = nc.dram_tensor("wvT", [128, CCH * C], BF16, kind="ExternalInput")
    bones = nc.dram_tensor("bones", [128, 128], BF16, kind="ExternalInput")
    yb = nc.dram_tensor("yb", [NCH * 128, CCH * NW], F32,
                        kind="ExternalOutput")

    with tile.TileContext(nc) as tc:
        _body(tc, xbbf, xq, ftT, wqr, wp8, wkT, wvT, bones, yb)
    nc.compile()
    return nc


def _body(tc, xbbf, xq, ftT, wqr, wp8, wkT, wvT, bones, yb):
    nc = tc.nc
    Exp = mybir.ActivationFunctionType.Exp

    with (
        tc.tile_pool(name="const", bufs=1) as const,
        tc.tile_pool(name="xbf", bufs=4) as xbfp,
        tc.tile_pool(name="xq8", bufs=3) as xqp,
        tc.tile_pool(name="qt", bufs=2) as qtp,
        tc.tile_pool(name="expt", bufs=3) as expp,
        tc.tile_pool(name="rcp", bufs=2) as rcp,
        tc.tile_pool(name="ycorr", bufs=2) as ycp,
        tc.tile_pool(name="out8", bufs=2) as outp,
        tc.tile_pool(name="yout", bufs=2) as yop,
        tc.tile_pool(name="ps_qy", bufs=2, space="PSUM") as ps_qy,
        tc.tile_pool(name="ps_s", bufs=2, space="PSUM") as ps_s,
        tc.tile_pool(name="ps_sb", bufs=2, space="PSUM") as ps_sb,
        tc.tile_pool(name="ps_o", bufs=2, space="PSUM") as ps_o,
    ):
        # ---- input loaders; x8 on the critical path, xbf deferred ----------
        hist = {}

        def load_x(c):
            st = {"i": c}
            t = xqp.tile([128, 2, 2, NW], FP8, name="x8_t", tag="x8")
            nc.sync.dma_start(out=t[:], in_=xq[128 * c:128 * (c + 1), :])
            st["x8"] = t
            st["qT"] = [None] * CCH
            st["expT"] = [None] * CCH
            st["rb"] = [None] * CCH
            return st

        def load_xbf(st):
            st["xbf"] = xbfp.tile([128, CCH, NW], BF16, name="xbf_t", tag="xbf")
            c = st["i"]
            nc.sync.dma_start(out=st["xbf"][:],
                              in_=xbbf[128 * c:128 * (c + 1), :])

        # DMA issue order = priority order: chunk-0 q inputs first, then the
        # k/v-projection weights, then everything else.
        ftT_t = const.tile([128, CCH, K], BF16, tag="ftT")
        nc.sync.dma_start(out=ftT_t[:], in_=ftT[:, :])
        wk_t = const.tile([128, CCH, C], BF16, tag="wkT")
        nc.sync.dma_start(out=wk_t[:], in_=wkT[:, :])
        wq_t = const.tile([128, CCH, C], BF16, tag="wqr")
        nc.sync.dma_start(out=wq_t[:], in_=wqr[:, :])
        hist[0] = load_x(0)
        hist[1] = load_x(1)
        wv_t = const.tile([128, CCH, C], BF16, tag="wvT")
        nc.sync.dma_start(out=wv_t[:], in_=wvT[:, :])
        bones_sb = const.tile([128, 128], BF16, tag="bones")
        nc.sync.dma_start(out=bones_sb[:], in_=bones[:, :])
        wp_t = const.tile([128, 2, 2, C], FP8, tag="wp8")
        nc.sync.dma_start(out=wp_t[:], in_=wp8[:, :])
        load_xbf(hist[0])

        # ---- pipeline stage helpers ----------------------------------------
        kbd = []
        vbd = []
        m8 = []

        def s_stage(st, j):
            ps = ps_s.tile([128, NW], F32, name="ps", tag="ps")
            for cb in range(2):
                nc.tensor.matmul(
                    ps[:],
                    lhsT=m8[j][cb][:],
                    rhs=st["x8"][:, cb, :, :],
                    start=(cb == 0),
                    stop=(cb == 1),
                    perf_mode=DR,
                )
            t = expp.tile([128, NW], BF16, name="expT_t", tag=f"e{j}")
            nc.scalar.activation(t[:], ps[:], Exp, bias=0.0,
                                 scale=0.125 / SQ2)
            st["expT"][j] = t

        def sb_stage(st, j):
            pb = ps_sb.tile([128, NW], F32, name="pb", tag="pb")
            nc.tensor.matmul(pb[:], lhsT=bones_sb[:], rhs=st["expT"][j][:],
                             start=True, stop=True)
            t = rcp.tile([128, NW], F32, name="rb_t", tag=f"r{j}")
            nc.vector.reciprocal_approx_fast(t[:], pb[:])
            st["rb"][j] = t

        def out_stage(st, j):
            if j == 0:
                st["o8"] = [
                    outp.tile([128, 2, NW], FP8, name="o8_t", tag=f"o8{cb}")
                    for cb in range(2)
                ]
                st["yo"] = yop.tile([128, CCH, NW], F32, name="yo_t", tag="yo")
            po = ps_o.tile([128, NW], F32, name="po", tag="po")
            nc.tensor.matmul(po[:], lhsT=vbd[j][:], rhs=st["expT"][j][:],
                             start=True, stop=True)
            with nc.allow_low_precision(reason="fp8 attention out tile"):
                nc.vector.tensor_mul(st["o8"][j // 2][:, j % 2, :],
                                     po[:], st["rb"][j][:])

        def y_group(st, m):
            py = ps_qy.tile([128, NW], F32, name="py", tag="qy")
            for cb in range(2):
                nc.tensor.matmul(
                    py[:],
                    lhsT=wp_t[:, cb, :, 128 * m:128 * (m + 1)],
                    rhs=st["o8"][cb][:],
                    start=(cb == 0),
                    stop=(cb == 1),
                    perf_mode=DR,
                )
            if st["i"] < NCH - 1:
                yc = ycp.tile([128, NW], BF16, name="yc_t", tag=f"yc{m % 2}")
                with nc.allow_low_precision(reason="bf16 projection tail"):
                    nc.scalar.activation(
                        yc[:], py[:], mybir.ActivationFunctionType.Copy,
                        bias=0.0, scale=YSCALE,
                    )
                nc.gpsimd.tensor_add(st["yo"][:, m, :], yc[:],
                                     st["xbf"][:, m, :])
            else:
                nc.vector.affine_then_add(
                    st["yo"][:, m, :], py[:], st["xbf"][:, m, :],
                    scale=YSCALE, bias=0.0,
                )
            if m % 2 == 1:
                c = st["i"]
                nc.sync.dma_start(
                    out=yb[128 * c:128 * (c + 1), NW * (m - 1):NW * (m + 1)],
                    in_=st["yo"][:, m - 1:m + 1, :],
                )

        # ---- kT = Wk @ Ft^T, packed block-diagonal per head pair -----------
        for cj in range(CCH):
            t = const.tile([128, 128], BF16, tag=f"kbd{cj}")
            nc.vector.memset(t[:], 0.0)
            kbd.append(t)
        for cj in range(CCH):
            pk = ps_s.tile([128, NW], F32, tag="ps")
            for mk in range(CCH):
                nc.tensor.matmul(
                    pk[:, :K],
                    lhsT=wk_t[:, mk, 128 * cj:128 * (cj + 1)],
                    rhs=ftT_t[:, mk, :],
                    start=(mk == 0),
                    stop=(mk == CCH - 1),
                )
            nc.vector.tensor_copy(kbd[cj][0:64, 0:64], pk[0:64, :K])
            nc.vector.tensor_copy(kbd[cj][64:128, 64:128], pk[64:128, :K])

        # ---- fused scores weights: M_jT = Wq_j.T @ kbd_j, cast to fp8 DR ---
        for j in range(CCH):
            m8.append([
                const.tile([128, 2, 128], FP8, name=f"m8_{j}_{cb}",
                           tag=f"m8_{j}_{cb}")
                for cb in range(2)
            ])
        for j in range(CCH):
            pm = ps_sb.tile([128, NW], F32, tag="pb")
            for cc in range(CCH):
                nc.tensor.matmul(
                    pm[:, 128 * cc:128 * (cc + 1)],
                    lhsT=wq_t[:, j, 128 * cc:128 * (cc + 1)],
                    rhs=kbd[j][:],
                    start=True,
                    stop=True,
                )
            with nc.allow_low_precision(reason="fp8 fused score weights"):
                for cb in range(2):
                    for i in range(2):
                        nc.scalar.activation(
                            m8[j][cb][:, i, :],
                            pm[:, 128 * (2 * cb + i):128 * (2 * cb + i + 1)],
                            mybir.ActivationFunctionType.Copy,
                            bias=0.0, scale=SQ2,
                        )

        # ---- v = Ft @ Wv^T [K, C], duplicated then packed block-diag -------
        v_dup = const.tile([128, C], BF16, tag="vdup")
        pv = ps_o.tile([128, NW], F32, tag="po")
        for mk in range(CCH):
            nc.tensor.matmul(
                pv[0:64, :],
                lhsT=ftT_t[:, mk, :],
                rhs=wv_t[:, mk, :],
                start=(mk == 0),
                stop=(mk == CCH - 1),
            )
        nc.vector.tensor_copy(v_dup[0:64, :], pv[0:64, :])
        nc.sync.dma_start(out=v_dup[64:128, :], in_=v_dup[0:64, :])
        for cj in range(CCH):
            t = const.tile([128, 128], BF16, tag=f"vbd{cj}")
            nc.vector.memset(t[:], 0.0)
            vbd.append(t)
        for cj in range(CCH):
            nc.vector.tensor_copy(vbd[cj][0:64, 0:64],
                                  v_dup[0:64, 128 * cj:128 * cj + 64])
            nc.vector.tensor_copy(vbd[cj][64:128, 64:128],
                                  v_dup[64:128, 128 * cj + 64:128 * cj + 128])

        # ---- main loop: 4-stream round-robin software pipeline -------------
        # iteration t runs: q(t), scores(t-1), sums+out(t-2), y(t-3); adjacent
        # PE groups always come from different streams, so every semaphore
        # wait is covered by independent matmul work and the PE never idles
        # long enough to drop the HAM clock.
        for t in range(NCH + 2):
            if 2 <= t + 2 < NCH:
                hist[t + 2] = load_x(t + 2)
            if 1 <= t + 1 < NCH:
                load_xbf(hist[t + 1])
            ss = hist.get(t) if t < NCH else None
            bo = hist.get(t - 1)
            ys = hist.get(t - 2)
            for r in range(CCH):
                if ys is not None:
                    y_group(ys, r)
                if ss is not None:
                    s_stage(ss, r)
                if bo is not None:
                    sb_stage(bo, r)
                    out_stage(bo, r)
            if ys is not None:
                del hist[t - 2]


_NC_CACHE = None
LAST_RESULTS = None


def kernel(x, Ft, Wq, Wk, Wv, Wp, bp):
    global _NC_CACHE, LAST_RESULTS
    import ml_dtypes

    bf16 = ml_dtypes.bfloat16
    e4 = ml_dtypes.float8_e4m3

    def toe4(a):
        return np.clip(a, -240.0, 240.0).astype(e4)

    x = np.ascontiguousarray(np.asarray(x, dtype=np.float32))
    Ft = np.asarray(Ft, dtype=np.float32)

    # fp8 weights, one [128, cb, i, o] block: c = cb*256 + i*128 + p
    def pack_dr(WT_scaled):
        a = WT_scaled.reshape(2, 2, 128, C).transpose(2, 0, 1, 3)
        return np.ascontiguousarray(a.reshape(128, 4 * C))

    wp8 = toe4(pack_dr(np.asarray(Wp, dtype=np.float32).T * SP))
    wqr = np.ascontiguousarray(
        np.asarray(Wq, dtype=np.float32).reshape(CCH, 128, C)
        .transpose(1, 0, 2).reshape(128, CCH * C)).astype(bf16)
    # bf16 weights [128, mk, c]: row p, slot mk holds W.T[mk*128+p, c]
    wkT = np.ascontiguousarray(
        np.asarray(Wk, dtype=np.float32).T.reshape(CCH, 128, C)
        .transpose(1, 0, 2).reshape(128, CCH * C)).astype(bf16)
    wvT = np.ascontiguousarray(
        np.asarray(Wv, dtype=np.float32).T.reshape(CCH, 128, C)
        .transpose(1, 0, 2).reshape(128, CCH * C)).astype(bf16)
    ftT = np.ascontiguousarray(
        Ft.transpose(0, 2, 1).reshape(B, CCH, 128, K)
        .transpose(0, 2, 1, 3).reshape(B, 128, CCH * K)).astype(bf16)

    bones = np.zeros((128, 128), dtype=np.float32)
    bones[0:64, 0:64] = 1.0 / SO
    bones[64:128, 64:128] = 1.0 / SO
    bones = bones.astype(bf16)

    xr = x.reshape(B, C, N)
    # residual stream, chunk-major [ch*128+p, j*NW+nw] with bp folded in
    xrbf = (xr + np.asarray(bp, dtype=np.float32).reshape(1, C, 1)).astype(bf16)
    xrbf = xrbf.reshape(B, CCH, 128, NCH, NW).transpose(0, 3, 2, 1, 4)
    xrbf = np.ascontiguousarray(xrbf.reshape(B, NCH * 128, CCH * NW))
    # x fp8 DoubleRow chunk-major layout [ch*128+p, ((cb*2+i)*NW)+nw]
    x8 = toe4(xr).reshape(B, 2, 2, 128, NCH, NW).transpose(0, 4, 3, 1, 2, 5)
    x8 = np.ascontiguousarray(x8.reshape(B, NCH * 128, 4 * NW))

    if _NC_CACHE is None:
        _NC_CACHE = build_bass()
    nc = _NC_CACHE

    in_maps = [
        {
            "xbbf": xrbf[b],
            "xq": x8[b],
            "ftT": ftT[b],
            "wqr": wqr,
            "wp8": wp8,
            "wkT": wkT,
            "wvT": wvT,
            "bones": bones,
        }
        for b in range(B)
    ]
    res = run_bass_kernel_spmd(nc, in_maps, core_ids=list(range(N_CORES)))
    LAST_RESULTS = res
    y = np.stack([res.results[b]["yb"] for b in range(B)])
    # y chunk-major [ch*128+p, m*NW+nw] -> [c = m*128+p, n = ch*NW+nw]
    y = y.reshape(B, NCH, 128, CCH, NW).transpose(0, 3, 2, 1, 4)
    return np.ascontiguousarray(y.reshape(B, C, HW, HW))


# revision 23
# speedup vs baseline: 1.2869x; 1.0275x over previous
"""Trainium2 Bass kernel for nn_FMG_6717328851807 (dense_transformer).

Reference computation (B=8, C=512, H=W=64, K=64, MEM=512, heads=8, d=64):
    q = Wq @ x            (1x1 conv)          -> [B,h,N,d], N = H*W = 4096
    k = Ft @ Wk.T, v = Ft @ Wv.T              -> [B,h,K,d]
    attn = softmax(q k^T / sqrt(d))           -> [B,h,N,K]
    out = attn @ v                            -> [B,h,N,d]
    y = x + Wp @ out + bp

Sharding: pure data-parallel over B - one batch element per NeuronCore,
no collectives. Within a core everything runs transposed (channels on
partitions, spatial N on the free dim) in NW=512-column chunks, and
every PE op is a full-width matmul:

    qT[C,N]    = Wq8.T @ x8            fp8e4m3 DoubleRow: 2 MMs of
                                       256-deep contraction per 128-row
                                       group (Wq pre-scaled by 64 on the
                                       host; the 1/64 folds into the
                                       softmax exp scale)
    kT, v      = bf16 setup matmuls, then repacked into BLOCK-DIAGONAL
                 [128,128] tiles per head pair so that
    scoresT    = kbd.T @ qT            one 128-wide MM per head pair
    expT       = exp(scoresT/(8*64))   ScalarE, bf16 out
    sums_bc    = blockones.T @ expT    one MM per pair computes the
                                       softmax denominator AND
                                       broadcasts it to all 64
                                       partitions of its head
                                       (blockones holds 1/64, so the
                                       reciprocal is pre-scaled for the
                                       fp8 out tile)
    rb         = recip_approx(sums_bc) DVE, fp32
    outT8      = po * rb               DVE multiply, written straight
                                       into the fp8 DoubleRow moving
                                       layout for the y projection
    y          = (Wp8.T @ outT8)/4096 + (x + bp)
                                       2 DoubleRow MMs per 128-row
                                       group; ScalarE+GpSimd (m<2) or a
                                       fused DVE affine_then_add (m>=2)
                                       do the rescale + bf16 residual.

All HBM traffic is chunk-major so each chunk moves with ONE descriptor-
friendly >=256KB dma_start per stream (inputs x8/xbf, half-chunk y
stores); weights load as single whole-tensor transfers. The host packs
the inputs into these layouts and unscrambles the chunk-major output.
Chunk-0's q projection runs first and doubles as the PE HAM warm-up.
"""

import numpy as np

import concourse.bass as bass
import concourse.mybir as mybir
import concourse.tile as tile
from concourse import bacc
from concourse.bass_utils import run_bass_kernel_spmd

F32 = mybir.dt.float32
BF16 = mybir.dt.bfloat16
FP8 = mybir.dt.float8e4
DR = mybir.MatmulPerfMode.DoubleRow

B, C, N = 8, 512, 4096
HW = 64
K, MEM, H, D = 64, 512, 8, 64
NW = 512                # columns of N processed per chunk
NCH = N // NW           # 8 chunks
CCH = C // 128          # 4 chunks of channels/partitions
N_CORES = 8
SQ2 = 128.0             # on-chip scale on the fused score weights
SO = 64.0               # on-chip scale on outT (via blockones=1/64)
SP = 64.0               # host pre-scale on Wp
YSCALE = 1.0 / (SO * SP)


def build_bass():
    nc = bacc.Bacc("TRN2", target_bir_lowering=False, debug=False)

    # chunk-major input/output layouts; one dma_start per chunk per stream
    xbbf = nc.dram_tensor("xbbf", [NCH * 128, CCH * NW], BF16,
                          kind="ExternalInput")
    xq = nc.dram_tensor("xq", [NCH * 128, 4 * NW], FP8, kind="ExternalInput")
    ftT = nc.dram_tensor("ftT", [128, CCH * K], BF16, kind="ExternalInput")
    wqr = nc.dram_tensor("wqr", [128, CCH * C], FP8, kind="ExternalInput")
    wp8 = nc.dram_tensor("wp8", [128, 4 * C], FP8, kind="ExternalInput")
    wkT = nc.dram_tensor("wkT", [128, CCH * C], FP8, kind="ExternalInput")
    wvT = nc.dram_tensor("wvT", [128, CCH * C], BF16, kind="ExternalInput")
    bones = nc.dram_tensor("bones", [128, 128], BF16, kind="ExternalInput")
    yb = nc.dram_tensor("yb", [NCH * 128, CCH * NW], BF16,
                        kind="ExternalOutput")

    with tile.TileContext(nc) as tc:
        _body(tc, xbbf, xq, ftT, wqr, wp8, wkT, wvT, bones, yb)
    nc.compile()
    return nc


def _body(tc, xbbf, xq, ftT, wqr, wp8, wkT, wvT, bones, yb):
    nc = tc.nc
    Exp = mybir.ActivationFunctionType.Exp

    with (
        tc.tile_pool(name="const", bufs=1) as const,
        tc.tile_pool(name="xbf", bufs=4) as xbfp,
        tc.tile_pool(name="xq8", bufs=3) as xqp,
        tc.tile_pool(name="qt", bufs=2) as qtp,
        tc.tile_pool(name="expt", bufs=3) as expp,
        tc.tile_pool(name="rcp", bufs=2) as rcp,
        tc.tile_pool(name="ycorr", bufs=2) as ycp,
        tc.tile_pool(name="out8", bufs=2) as outp,
        tc.tile_pool(name="yout", bufs=2) as yop,
        tc.tile_pool(name="ps_qy", bufs=2, space="PSUM") as ps_qy,
        tc.tile_pool(name="ps_s", bufs=2, space="PSUM") as ps_s,
        tc.tile_pool(name="ps_sb", bufs=2, space="PSUM") as ps_sb,
        tc.tile_pool(name="ps_o", bufs=2, space="PSUM") as ps_o,
    ):
        # ---- input loaders; x8 on the critical path, xbf deferred ----------
        hist = {}

        def load_x(c):
            st = {"i": c}
            t = xqp.tile([128, 2, 2, NW], FP8, name="x8_t", tag="x8")
            nc.sync.dma_start(out=t[:], in_=xq[128 * c:128 * (c + 1), :])
            st["x8"] = t
            st["qT"] = [None] * CCH
            st["expT"] = [None] * CCH
            st["rb"] = [None] * CCH
            return st

        def load_xbf(st):
            st["xbf"] = xbfp.tile([128, CCH, NW], BF16, name="xbf_t", tag="xbf")
            c = st["i"]
            nc.sync.dma_start(out=st["xbf"][:],
                              in_=xbbf[128 * c:128 * (c + 1), :])

        # DMA issue order = priority order: chunk-0 q inputs first, then the
        # k/v-projection weights, then everything else.
        ftT_t = const.tile([128, CCH, K], BF16, tag="ftT")
        nc.sync.dma_start(out=ftT_t[:], in_=ftT[:, :])
        wk_t = const.tile([128, CCH, C], FP8, tag="wkT")
        nc.sync.dma_start(out=wk_t[:], in_=wkT[:, :])
        wq_t = const.tile([128, CCH, C], FP8, tag="wqr")
        nc.sync.dma_start(out=wq_t[:], in_=wqr[:, :])
        hist[0] = load_x(0)
        hist[1] = load_x(1)
        wv_t = const.tile([128, CCH, C], BF16, tag="wvT")
        nc.sync.dma_start(out=wv_t[:], in_=wvT[:, :])
        bones_sb = const.tile([128, 128], BF16, tag="bones")
        nc.sync.dma_start(out=bones_sb[:], in_=bones[:, :])
        wp_t = const.tile([128, 2, 2, C], FP8, tag="wp8")
        nc.sync.dma_start(out=wp_t[:], in_=wp8[:, :])
        load_xbf(hist[0])

        # ---- pipeline stage helpers ----------------------------------------
        kbd = []
        vbd = []
        m8 = []

        def s_stage(st, j):
            ps = ps_s.tile([128, NW], F32, name="ps", tag="ps")
            for cb in range(2):
                nc.tensor.matmul(
                    ps[:],
                    lhsT=m8[j][cb][:],
                    rhs=st["x8"][:, cb, :, :],
                    start=(cb == 0),
                    stop=(cb == 1),
                    perf_mode=DR,
                )
            t = expp.tile([128, NW], BF16, name="expT_t", tag=f"e{j}")
            nc.scalar.activation(t[:], ps[:], Exp, bias=0.0,
                                 scale=0.125 / SQ2)
            st["expT"][j] = t

        def sb_stage(st, j):
            pb = ps_sb.tile([128, NW], F32, name="pb", tag="pb")
            nc.tensor.matmul(pb[:], lhsT=bones_sb[:], rhs=st["expT"][j][:],
                             start=True, stop=True)
            t = rcp.tile([128, NW], F32, name="rb_t", tag=f"r{j}")
            nc.vector.reciprocal_approx_fast(t[:], pb[:])
            st["rb"][j] = t

        def out_stage(st, j):
            if j == 0:
                st["o8"] = [
                    outp.tile([128, 2, NW], FP8, name="o8_t", tag=f"o8{cb}")
                    for cb in range(2)
                ]
                st["yo"] = yop.tile([128, CCH, NW], BF16, name="yo_t", tag="yo")
            po = ps_o.tile([128, NW], F32, name="po", tag="po")
            nc.tensor.matmul(po[:], lhsT=vbd[j][:], rhs=st["expT"][j][:],
                             start=True, stop=True)
            with nc.allow_low_precision(reason="fp8 attention out tile"):
                nc.vector.tensor_mul(st["o8"][j // 2][:, j % 2, :],
                                     po[:], st["rb"][j][:])

        def y_group(st, m):
            py = ps_qy.tile([128, NW], F32, name="py", tag="qy")
            for cb in range(2):
                nc.tensor.matmul(
                    py[:],
                    lhsT=wp_t[:, cb, :, 128 * m:128 * (m + 1)],
                    rhs=st["o8"][cb][:],
                    start=(cb == 0),
                    stop=(cb == 1),
                    perf_mode=DR,
                )
            if st["i"] < NCH - 1:
                yc = ycp.tile([128, NW], BF16, name="yc_t", tag=f"yc{m % 2}")
                with nc.allow_low_precision(reason="bf16 projection tail"):
                    nc.scalar.activation(
                        yc[:], py[:], mybir.ActivationFunctionType.Copy,
                        bias=0.0, scale=YSCALE,
                    )
                with nc.allow_low_precision(reason="bf16 output"):
                    nc.gpsimd.tensor_add(st["yo"][:, m, :], yc[:],
                                         st["xbf"][:, m, :])
            else:
                with nc.allow_low_precision(reason="bf16 output"):
                    nc.vector.affine_then_add(
                        st["yo"][:, m, :], py[:], st["xbf"][:, m, :],
                        scale=YSCALE, bias=0.0,
                    )
            if m % 2 == 1:
                c = st["i"]
                nc.sync.dma_start(
                    out=yb[128 * c:128 * (c + 1), NW * (m - 1):NW * (m + 1)],
                    in_=st["yo"][:, m - 1:m + 1, :],
                )

        # ---- kT = Wk @ Ft^T, packed block-diagonal per head pair -----------
        for cj in range(CCH):
            t = const.tile([128, 128], BF16, tag=f"kbd{cj}")
            nc.vector.memset(t[:], 0.0)
            kbd.append(t)
        for cj in range(CCH):
            pk = ps_s.tile([128, NW], F32, tag="ps")
            for mk in range(CCH):
                nc.tensor.matmul(
                    pk[:, :K],
                    lhsT=wk_t[:, mk, 128 * cj:128 * (cj + 1)],
                    rhs=ftT_t[:, mk, :],
                    start=(mk == 0),
                    stop=(mk == CCH - 1),
                )
            nc.vector.tensor_copy(kbd[cj][0:64, 0:64], pk[0:64, :K])
            nc.vector.tensor_copy(kbd[cj][64:128, 64:128], pk[64:128, :K])

        # ---- fused scores weights: M_jT = Wq_j.T @ kbd_j, cast to fp8 DR ---
        for j in range(CCH):
            m8.append([
                const.tile([128, 2, 128], FP8, name=f"m8_{j}_{cb}",
                           tag=f"m8_{j}_{cb}")
                for cb in range(2)
            ])
        for j in range(CCH):
            pm = ps_sb.tile([128, NW], F32, tag="pb")
            for cc in range(CCH):
                nc.tensor.matmul(
                    pm[:, 128 * cc:128 * (cc + 1)],
                    lhsT=wq_t[:, j, 128 * cc:128 * (cc + 1)],
                    rhs=kbd[j][:],
                    start=True,
                    stop=True,
                )
            with nc.allow_low_precision(reason="fp8 fused score weights"):
                for cb in range(2):
                    for i in range(2):
                        nc.scalar.activation(
                            m8[j][cb][:, i, :],
                            pm[:, 128 * (2 * cb + i):128 * (2 * cb + i + 1)],
                            mybir.ActivationFunctionType.Copy,
                            bias=0.0, scale=1.0 / 32.0,
                        )

        # ---- v = Ft @ Wv^T [K, C], duplicated then packed block-diag -------
        v_dup = const.tile([128, C], BF16, tag="vdup")
        pv = ps_o.tile([128, NW], F32, tag="po")
        for mk in range(CCH):
            nc.tensor.matmul(
                pv[0:64, :],
                lhsT=ftT_t[:, mk, :],
                rhs=wv_t[:, mk, :],
                start=(mk == 0),
                stop=(mk == CCH - 1),
            )
        nc.vector.tensor_copy(v_dup[0:64, :], pv[0:64, :])
        nc.sync.dma_start(out=v_dup[64:128, :], in_=v_dup[0:64, :])
        for cj in range(CCH):
            t = const.tile([128, 128], BF16, tag=f"vbd{cj}")
            nc.vector.memset(t[:], 0.0)
            vbd.append(t)
        for cj in range(CCH):
            nc.vector.tensor_copy(vbd[cj][0:64, 0:64],
                                  v_dup[0:64, 128 * cj:128 * cj + 64])
            nc.vector.tensor_copy(vbd[cj][64:128, 64:128],
                                  v_dup[64:128, 128 * cj + 64:128 * cj + 128])

        # ---- main loop: 4-stream round-robin software pipeline -------------
        # iteration t runs: q(t), scores(t-1), sums+out(t-2), y(t-3); adjacent
        # PE groups always come from different streams, so every semaphore
        # wait is covered by independent matmul work and the PE never idles
        # long enough to drop the HAM clock.
        for t in range(NCH + 2):
            if 2 <= t + 2 < NCH:
                hist[t + 2] = load_x(t + 2)
            if 1 <= t + 1 < NCH:
                load_xbf(hist[t + 1])
            ss = hist.get(t) if t < NCH else None
            bo = hist.get(t - 1)
            ys = hist.get(t - 2)
            for r in range(CCH):
                if ys is not None:
                    y_group(ys, r)
                if ss is not None:
                    s_stage(ss, r)
                if bo is not None:
                    sb_stage(bo, r)
                    out_stage(bo, r)
            if ys is not None:
                del hist[t - 2]


_NC_CACHE = None
LAST_RESULTS = None


def kernel(x, Ft, Wq, Wk, Wv, Wp, bp):
    global _NC_CACHE, LAST_RESULTS
    import ml_dtypes

    bf16 = ml_dtypes.bfloat16
    e4 = ml_dtypes.float8_e4m3

    def toe4(a):
        return np.clip(a, -240.0, 240.0).astype(e4)

    x = np.ascontiguousarray(np.asarray(x, dtype=np.float32))
    Ft = np.asarray(Ft, dtype=np.float32)

    # fp8 weights, one [128, cb, i, o] block: c = cb*256 + i*128 + p
    def pack_dr(WT_scaled):
        a = WT_scaled.reshape(2, 2, 128, C).transpose(2, 0, 1, 3)
        return np.ascontiguousarray(a.reshape(128, 4 * C))

    wp8 = toe4(pack_dr(np.asarray(Wp, dtype=np.float32).T * SP))
    wqr = toe4(np.ascontiguousarray(
        64.0 * np.asarray(Wq, dtype=np.float32).reshape(CCH, 128, C)
        .transpose(1, 0, 2).reshape(128, CCH * C)))
    # bf16 weights [128, mk, c]: row p, slot mk holds W.T[mk*128+p, c]
    wkT = toe4(np.ascontiguousarray(
        64.0 * np.asarray(Wk, dtype=np.float32).T.reshape(CCH, 128, C)
        .transpose(1, 0, 2).reshape(128, CCH * C)))
    wvT = np.ascontiguousarray(
        np.asarray(Wv, dtype=np.float32).T.reshape(CCH, 128, C)
        .transpose(1, 0, 2).reshape(128, CCH * C)).astype(bf16)
    ftT = np.ascontiguousarray(
        Ft.transpose(0, 2, 1).reshape(B, CCH, 128, K)
        .transpose(0, 2, 1, 3).reshape(B, 128, CCH * K)).astype(bf16)

    bones = np.zeros((128, 128), dtype=np.float32)
    bones[0:64, 0:64] = 1.0 / SO
    bones[64:128, 64:128] = 1.0 / SO
    bones = bones.astype(bf16)

    xr = x.reshape(B, C, N)
    # residual stream, chunk-major [ch*128+p, j*NW+nw] with bp folded in
    xrbf = (xr + np.asarray(bp, dtype=np.float32).reshape(1, C, 1)).astype(bf16)
    xrbf = xrbf.reshape(B, CCH, 128, NCH, NW).transpose(0, 3, 2, 1, 4)
    xrbf = np.ascontiguousarray(xrbf.reshape(B, NCH * 128, CCH * NW))
    # x fp8 DoubleRow chunk-major layout [ch*128+p, ((cb*2+i)*NW)+nw]
    x8 = toe4(xr).reshape(B, 2, 2, 128, NCH, NW).transpose(0, 4, 3, 1, 2, 5)
    x8 = np.ascontiguousarray(x8.reshape(B, NCH * 128, 4 * NW))

    if _NC_CACHE is None:
        _NC_CACHE = build_bass()
    nc = _NC_CACHE

    in_maps = [
        {
            "xbbf": xrbf[b],
            "xq": x8[b],
            "ftT": ftT[b],
            "wqr": wqr,
            "wp8": wp8,
            "wkT": wkT,
            "wvT": wvT,
            "bones": bones,
        }
        for b in range(B)
    ]
    res = run_bass_kernel_spmd(nc, in_maps, core_ids=list(range(N_CORES)))
    LAST_RESULTS = res
    y = np.stack([np.asarray(res.results[b]["yb"], dtype=np.float32)
                  for b in range(B)])
    # y chunk-major [ch*128+p, m*NW+nw] -> [c = m*128+p, n = ch*NW+nw]
    y = y.reshape(B, NCH, 128, CCH, NW).transpose(0, 3, 2, 1, 4)
    return np.ascontiguousarray(y.reshape(B, C, HW, HW))
